# revision 1
# baseline (speedup 1.0000x reference)
"""Causal self-attention (B=2, T=2048, C=1024, 16 heads x 64) on 8 TRN2 cores.

Sharding: tensor-parallel over heads (2 heads/core). Each core computes its
heads' QKV projection, causal attention, and a partial output projection
(contraction over its 128 attn columns); the host sums the 8 partials
(row-parallel all-reduce at gather time).

Per-core kernel layout (v2, PE-warmth/weight-reuse optimized):
  - x pre-transposed on host to xT [ci=128, co=8, B*T] (c = co*128+ci).
  - qT/kT/vT [f, t] computed with c-outer loops (stationary weight reused
    across 4 moving chunks); V additionally PE-transposed to t-major with
    an appended ones column.
  - Scores computed transposed, ST[k, q] = KT^T @ QT; the two heads' K=64
    matmuls are emitted alternately so they row-pack into the 128x128 PE
    array concurrently.
  - exp via one ACT pass per [128, 2, 512] group, PSUM->SBUF bf16.
  - No max-subtraction (scores ~N(0,1); exp safe in fp32).
  - PV accumulates [65, q]: V ones-column makes row 64 the softmax
    denominator l[q]. PV is ragged on the causal diagonal band.
  - Normalization (1/l) via reciprocal_approx_fast + K=2-style broadcast
    matmul (sel65) + one DVE multiply into attnT.
  - Output projection per q-chunk right after normalization (keeps PE
    busy through phase transitions); emits out[t, co] fp32 partials.
"""

import os

import numpy as np
import ml_dtypes

B = 2
T = 2048
C = 1024
N_HEADS = 16
D = 64
NCORES = 8
P = 128
BT = B * T
SCALE = D ** -0.5

_bf16 = ml_dtypes.bfloat16

_COMPILED = None
LAST_RESULTS = None  # stashed BassKernelResults for test harness introspection


def _build():
    import concourse.bass as bass
    import concourse.mybir as mybir
    import concourse.tile as tile
    from concourse import bacc

    f32 = mybir.dt.float32
    bf16 = mybir.dt.bfloat16

    nc = bacc.Bacc("TRN2", target_bir_lowering=False, debug=False,
                   num_devices=NCORES)

    xT_d = nc.dram_tensor("xT", [P, 8, BT], bf16, kind="ExternalInput")
    wqkvT_d = nc.dram_tensor("wqkvT", [P, 8, 384], bf16, kind="ExternalInput")
    woutT_d = nc.dram_tensor("woutT", [P, C], bf16, kind="ExternalInput")
    maskT_d = nc.dram_tensor("maskT", [P, P], bf16, kind="ExternalInput")
    sel2_d = nc.dram_tensor("sel2", [65, P], f32, kind="ExternalInput")
    ident_d = nc.dram_tensor("ident", [P, P], bf16, kind="ExternalInput")
    out_d = nc.dram_tensor("out", [BT, C], f32, kind="ExternalOutput")

    Exp = mybir.ActivationFunctionType.Exp

    with tile.TileContext(nc) as tc:
        with (
            tc.tile_pool(name="const", bufs=1) as const_pool,
            tc.tile_pool(name="xT", bufs=2) as xT_pool,
            tc.tile_pool(name="qkv", bufs=2) as qkv_pool,
            tc.tile_pool(name="pt", bufs=4) as pt_pool,
            tc.tile_pool(name="attnT", bufs=2) as attnT_pool,
            tc.tile_pool(name="rl", bufs=2) as rl_pool,
            tc.tile_pool(name="osb", bufs=3) as osb_pool,
            tc.tile_pool(name="st", bufs=2, space="PSUM") as st_pool,
            tc.tile_pool(name="ps4", bufs=4, space="PSUM") as ps4_pool,
        ):
            wqkvT = const_pool.tile([P, 8, 384], bf16, tag="wqkvT")
            woutT = const_pool.tile([P, C], bf16, tag="woutT")
            maskT = const_pool.tile([P, P], bf16, tag="maskT")
            sel2 = const_pool.tile([65, P], f32, tag="sel2")
            ident = const_pool.tile([P, P], bf16, tag="ident")
            nc.sync.dma_start(wqkvT, wqkvT_d[:])
            nc.sync.dma_start(woutT, woutT_d[:])
            nc.sync.dma_start(maskT, maskT_d[:])
            nc.sync.dma_start(sel2, sel2_d[:])
            nc.sync.dma_start(ident, ident_d[:])

            for b in range(B):
                xb = xT_pool.tile([P, 8, T], bf16, tag="xT")
                nc.sync.dma_start(xb, xT_d[:, :, b * T:(b + 1) * T])

                # ---- QKV projection: c-outer so the stationary weight is
                # reused across the 4 moving chunks of each projection.
                qT = qkv_pool.tile([P, T], bf16, tag="qT")
                kT = qkv_pool.tile([P, T], bf16, tag="kT")
                vT = qkv_pool.tile([P, T], bf16, tag="vT")
                for fi, dest in ((0, qT), (1, kT), (2, vT)):
                    pss = [ps4_pool.tile([P, 512], f32, tag="ps4",
                                         name=f"qkvps{n}")
                           for n in range(4)]
                    for c in range(8):
                        for n in range(4):
                            nc.tensor.matmul(
                                pss[n],
                                wqkvT[:, c, fi * 128:(fi + 1) * 128],
                                xb[:, c, n * 512:(n + 1) * 512],
                                start=(c == 0), stop=(c == 7),
                            )
                    for n in range(4):
                        nc.scalar.copy(dest[:, n * 512:(n + 1) * 512], pss[n])

                # V to t-major (PE transpose) with ones column appended.
                vh = [qkv_pool.tile([P, 16, 65], bf16, tag=f"v{h}",
                                    name=f"vh{h}")
                      for h in range(2)]
                for h in range(2):
                    nc.vector.memset(vh[h][:, :, 64], 1.0)
                for tch in range(16):
                    tp = ps4_pool.tile([P, P], bf16, tag="ps4", name="vtp")
                    nc.tensor.transpose(
                        tp, vT[:, tch * 128:(tch + 1) * 128], ident)
                    nc.scalar.copy(vh[0][:, tch, 0:64], tp[:, 0:64])
                    nc.scalar.copy(vh[1][:, tch, 0:64], tp[:, 64:128])

                # ---- attention (heads interleaved for PE row-packing) ----
                attnT = attnT_pool.tile([P, T], bf16, tag="attnT")
                rl2 = rl_pool.tile([65, T], f32, tag="rl2")
                l2 = rl_pool.tile([65, T], f32, tag="l2")
                # rows 1-63 feed zero sel2 rows; 1.0 keeps 1/x finite there
                nc.vector.memset(l2, 1.0)

                def norm_qc(qc):
                    # deps (recip) satisfied one qc ago -> no PE stall
                    qsl = slice(qc * 512, (qc + 1) * 512)
                    rb = ps4_pool.tile([P, 512], f32, tag="ps4", name="rb")
                    nc.tensor.matmul(rb, sel2[:, :], rl2[:, qsl],
                                     start=True, stop=True)
                    nc.vector.tensor_mul(attnT[:, qsl], attnT[:, qsl], rb)

                def oproj_tb(tb):
                    # PE filler: one token-block of the output projection
                    ps_a = ps4_pool.tile([P, 512], f32, tag="ps4",
                                         name="opa")
                    ps_b = ps4_pool.tile([P, 512], f32, tag="ps4",
                                         name="opb")
                    nc.tensor.matmul(
                        ps_a, attnT[:, tb * 128:(tb + 1) * 128],
                        woutT[:, 0:512], start=True, stop=True)
                    nc.tensor.matmul(
                        ps_b, attnT[:, tb * 128:(tb + 1) * 128],
                        woutT[:, 512:1024], start=True, stop=True)
                    osb = osb_pool.tile([P, C], f32, tag="osb")
                    nc.vector.tensor_copy(osb[:, 0:512], ps_a)
                    nc.vector.tensor_copy(osb[:, 512:1024], ps_b)
                    nc.sync.dma_start(
                        out_d[(b * T + tb * 128):(b * T + (tb + 1) * 128), :],
                        osb)

                for qc in range(4):
                    nk = 4 * qc + 4
                    qsl = slice(qc * 512, (qc + 1) * 512)
                    pv = [ps4_pool.tile([P, 512], f32, tag="ps4",
                                        name=f"pv{h}")
                          for h in range(2)]
                    if qc > 0:
                        norm_qc(qc - 1)
                    filler = list(range(4 * (qc - 1), 4 * qc)) if qc else []
                    for g0 in range(0, nk, 2):
                        kbs = list(range(g0, min(g0 + 2, nk)))
                        ng = len(kbs)
                        st = [st_pool.tile([P, 2, 512], f32, tag="st",
                                           name=f"st{h}")
                              for h in range(2)]
                        pt = [pt_pool.tile([P, 2, 512], bf16, tag="pt",
                                           name=f"pt{h}")
                              for h in range(2)]
                        # alternate heads so K=64 matmuls pack in the array
                        for j, kb in enumerate(kbs):
                            for h in range(2):
                                hs = h * 64
                                nc.tensor.matmul(
                                    st[h][:, j, :],
                                    kT[hs:hs + 64, kb * 128:(kb + 1) * 128],
                                    qT[hs:hs + 64, qsl],
                                    start=True, stop=True,
                                )
                        for h in range(2):
                            nc.scalar.activation(
                                pt[h][:, :ng, :], st[h][:, :ng, :], Exp,
                                scale=SCALE)
                        for j, kb in enumerate(kbs):
                            if kb >= 4 * qc:
                                off = (kb - 4 * qc) * 128
                                for h in range(2):
                                    nc.vector.tensor_mul(
                                        pt[h][:, j, off:off + 128],
                                        pt[h][:, j, off:off + 128],
                                        maskT,
                                    )
                        for j, kb in enumerate(kbs):
                            off = max(0, (kb - 4 * qc) * 128)
                            for h in range(2):
                                nc.tensor.matmul(
                                    pv[h][:65, off:512],
                                    vh[h][:, kb, :],
                                    pt[h][:, j, off:512],
                                    start=(kb == 0), stop=(kb == nk - 1),
                                    skip_group_check=True,
                                )
                        if filler:
                            oproj_tb(filler.pop(0))
                    # drain: denominators + unnormalized attnT.
                    # NOTE: custom-DVE ops (reciprocal_approx_*) mishandle
                    # non-zero partition bases on HW — move l to a base-0
                    # SBUF tile with regular copies first.
                    for h in range(2):
                        hs = h * 64
                        nc.vector.tensor_copy(
                            l2[hs:hs + 1, qsl], pv[h][64:65, :])
                        nc.vector.tensor_copy(
                            attnT[hs:hs + 64, qsl], pv[h][0:64, :])
                    nc.vector.reciprocal_approx_fast(
                        rl2[:, qsl], l2[:, qsl])
                    for tb in filler:
                        oproj_tb(tb)
                norm_qc(3)
                for tb in range(12, 16):
                    oproj_tb(tb)

    nc.compile()
    return nc


def _get_compiled():
    global _COMPILED
    if _COMPILED is None:
        _COMPILED = _build()
    return _COMPILED


def make_core_inputs(x, w_qkv, w_out):
    """Host-side shard prep: returns list of per-core input dicts."""
    xf = np.asarray(x, dtype=np.float32).reshape(BT, C)
    xT = np.ascontiguousarray(
        xf.T.reshape(8, P, BT).transpose(1, 0, 2)).astype(_bf16)

    maskT = np.zeros((P, P), dtype=_bf16)
    kk, qq = np.meshgrid(np.arange(P), np.arange(P), indexing="ij")
    maskT[kk <= qq] = 1.0

    sel2 = np.zeros((65, P), dtype=np.float32)
    sel2[0, 0:64] = 1.0
    sel2[64, 64:128] = 1.0

    ident = np.eye(P, dtype=_bf16)

    w_qkv = np.asarray(w_qkv, dtype=np.float32)
    w_out = np.asarray(w_out, dtype=np.float32)

    ins = []
    for core in range(NCORES):
        r0 = 2 * core * D
        wsel = np.concatenate([
            w_qkv[r0:r0 + 128],
            w_qkv[C + r0:C + r0 + 128],
            w_qkv[2 * C + r0:2 * C + r0 + 128],
        ], axis=0)  # [384, 1024]
        wqkvT = np.ascontiguousarray(
            wsel.T.reshape(8, P, 384).transpose(1, 0, 2)).astype(_bf16)
        woutT = np.ascontiguousarray(
            w_out[:, core * P:(core + 1) * P].T).astype(_bf16)
        ins.append({
            "xT": xT,
            "wqkvT": wqkvT,
            "woutT": woutT,
            "maskT": maskT,
            "sel2": sel2,
            "ident": ident,
        })
    return ins


def kernel(x, w_qkv, w_out):
    global LAST_RESULTS
    from concourse.bass_utils import run_bass_kernel_spmd

    nc = _get_compiled()
    ins = make_core_inputs(x, w_qkv, w_out)
    trace = bool(os.environ.get("KERNEL_TRACE"))
    res = run_bass_kernel_spmd(nc, ins, core_ids=list(range(NCORES)),
                               trace=trace)
    LAST_RESULTS = res
    out = np.zeros((BT, C), dtype=np.float32)
    for r in res.results:
        out += r["out"]
    return out.reshape(B, T, C)



# revision 17
# speedup vs baseline: 1.2404x; 1.2404x over previous
"""Causal self-attention (B=2, T=2048, C=1024, 16 heads x 64) on 8 TRN2 cores.

Sharding: tensor-parallel over heads (2 heads/core). Each core computes its
heads' QKV projection, causal attention, and a partial output projection
(contraction over its 128 attn columns); the host sums the 8 partials.

v3 (chunk-pipelined, engine-balanced):
  - x DMA'd per 512-token chunk; chunk n+1's Q/K/V projection matmuls are
    emitted as PE fillers inside attention qc=n, so the exp-gated gaps get
    PE work, the input DMA overlaps compute (no 27us head), and the PE
    never idles long enough for the HAM clock-throttle to re-arm.
  - Scores PSUM tiles are bf16 (1 bank), both heads in one tile per
    k-block; exp is trimmed to the live causal range on diagonal blocks.
  - Causal mask multiplies run on the (otherwise idle) GpSimd engine;
    all PSUM->SBUF evacuation copies run on Vector; Scalar does exp only.
  - Output projection is one N=1024 bf16 matmul per token block into a
    single PSUM bank + one Vector copy + bf16 DMA out (host sums in f32).
  - V transposed per chunk (PE transpose); both heads' slices land in one
    strided Vector copy per token block.
"""

import os
from collections import deque

import numpy as np
import ml_dtypes

B = 2
T = 2048
C = 1024
N_HEADS = 16
D = 64
NCORES = 8
P = 128
BT = B * T
SCALE = D ** -0.5
NCH = 4          # 512-token chunks per batch
CW = T // NCH    # chunk width (= qc width)

_bf16 = ml_dtypes.bfloat16

_COMPILED = None
LAST_RESULTS = None  # stashed BassKernelResults for test harness introspection


def _build():
    import concourse.bass as bass
    import concourse.mybir as mybir
    import concourse.tile as tile
    from concourse import bacc

    f32 = mybir.dt.float32
    bf16 = mybir.dt.bfloat16

    nc = bacc.Bacc("TRN2", target_bir_lowering=False, debug=False,
                   num_devices=NCORES)

    xT_d = nc.dram_tensor("xT", [P, 8, BT], bf16, kind="ExternalInput")
    wqkvT_d = nc.dram_tensor("wqkvT", [P, 8, 384], bf16, kind="ExternalInput")
    woutT_d = nc.dram_tensor("woutT", [P, C], bf16, kind="ExternalInput")
    maskT_d = nc.dram_tensor("maskT", [P, P], bf16, kind="ExternalInput")
    sel2_d = nc.dram_tensor("sel2", [65, P], f32, kind="ExternalInput")
    ident_d = nc.dram_tensor("ident", [P, P], bf16, kind="ExternalInput")
    out_d = nc.dram_tensor("out", [BT, C], bf16, kind="ExternalOutput")

    Exp = mybir.ActivationFunctionType.Exp

    with tile.TileContext(nc) as tc:
        with (
            tc.tile_pool(name="const", bufs=1) as const_pool,
            tc.tile_pool(name="xn", bufs=3) as xn_pool,
            tc.tile_pool(name="seq", bufs=2) as seq_pool,
            tc.tile_pool(name="vtn", bufs=2) as vtn_pool,
            tc.tile_pool(name="pt", bufs=4) as pt_pool,
            tc.tile_pool(name="osb", bufs=3) as osb_pool,
            tc.tile_pool(name="st", bufs=2, space="PSUM") as st_pool,
            tc.tile_pool(name="pv", bufs=2, space="PSUM") as pv_pool,
            tc.tile_pool(name="ps", bufs=2, space="PSUM") as ps_pool,
        ):
            wqkvT = const_pool.tile([P, 8, 384], bf16, tag="wqkvT")
            ident = const_pool.tile([P, P], bf16, tag="ident")
            maskT = const_pool.tile([P, P], bf16, tag="maskT")
            sel2 = const_pool.tile([65, P], f32, tag="sel2")
            woutT = const_pool.tile([P, C], bf16, tag="woutT")
            nc.sync.dma_start(wqkvT, wqkvT_d[:])
            nc.sync.dma_start(ident, ident_d[:])
            nc.sync.dma_start(maskT, maskT_d[:])
            nc.sync.dma_start(sel2, sel2_d[:])
            nc.sync.dma_start(woutT, woutT_d[:])

            fillers = deque()

            def drain(k):
                n = 0
                while fillers and n < k:
                    fillers.popleft()()
                    n += 1

            def flush():
                drain(len(fillers))

            def make_state(b):
                S = {}
                S["qT"] = seq_pool.tile([P, T], bf16, tag="qT",
                                        name=f"qT{b}")
                S["kT"] = seq_pool.tile([P, T], bf16, tag="kT",
                                        name=f"kT{b}")
                S["attnT"] = seq_pool.tile([P, T], bf16, tag="attnT",
                                           name=f"attnT{b}")
                S["vb"] = seq_pool.tile([P, 16, 2, 65], bf16, tag="vb",
                                        name=f"vb{b}")
                nc.vector.memset(S["vb"][:, :, :, 64], 1.0)
                S["l2"] = seq_pool.tile([65, T], f32, tag="l2",
                                        name=f"l2{b}")
                S["rl2"] = seq_pool.tile([65, T], f32, tag="rl2",
                                         name=f"rl2{b}")
                # rows 1-63 feed zero sel2 rows; 1.0 keeps 1/x finite there
                nc.vector.memset(S["l2"], 1.0)
                return S

            def make_chunk_quanta(b, n, S):
                """Queue chunk n's DMA now; return PE quanta closures."""
                xn = xn_pool.tile([P, 8, CW], bf16, tag="xn",
                                  name=f"x{b}_{n}")
                nc.sync.dma_start(
                    xn, xT_d[:, :, b * T + n * CW:b * T + (n + 1) * CW])
                nsl = slice(n * CW, (n + 1) * CW)
                ps_tiles = {}

                def proj_half(key, fsl, c0):
                    def f():
                        if c0 == 0:
                            ps_tiles[key] = ps_pool.tile(
                                [P, CW], f32, tag="ps",
                                name=f"ps{key}{b}{n}")
                        ps = ps_tiles[key]
                        for c in range(c0, c0 + 4):
                            nc.tensor.matmul(ps, wqkvT[:, c, fsl],
                                             xn[:, c, :],
                                             start=(c == 0), stop=(c == 7))
                    return f

                def v_fin():
                    vtn = vtn_pool.tile([P, CW], bf16, tag="vtn",
                                        name=f"vtn{b}{n}")
                    S["vtn"] = vtn
                    nc.vector.tensor_copy(vtn, ps_tiles["v"])

                def t_pair(j0):
                    def f():
                        for j in (j0, j0 + 1):
                            tp = ps_pool.tile([P, 2, 64], bf16, tag="ps",
                                              name=f"tp{b}{n}{j}")
                            nc.tensor.transpose(
                                tp[:, :, :],
                                S["vtn"][:, j * 128:(j + 1) * 128], ident)
                            nc.vector.tensor_copy(
                                S["vb"][:, 4 * n + j, :, 0:64], tp[:, :, :])
                    return f

                def q_fin():
                    nc.vector.tensor_copy(S["qT"][:, nsl], ps_tiles["q"])

                def k_fin():
                    nc.vector.tensor_copy(S["kT"][:, nsl], ps_tiles["k"])

                qv1 = proj_half("v", slice(256, 384), 0)
                qv2 = proj_half("v", slice(256, 384), 4)
                qq1 = proj_half("q", slice(0, 128), 0)
                qq2 = proj_half("q", slice(0, 128), 4)
                qk1 = proj_half("k", slice(128, 256), 0)
                qk2 = proj_half("k", slice(128, 256), 4)
                return [qv1, lambda: (qv2(), v_fin()), t_pair(0), t_pair(2),
                        qq1, lambda: (qq2(), q_fin()),
                        qk1, lambda: (qk2(), k_fin())]

            def emit_oproj(b, tb, S):
                osb = osb_pool.tile([P, C], bf16, tag="osb",
                                    name=f"osb{b}{tb}")
                for half in range(2):
                    csl = slice(half * 512, (half + 1) * 512)
                    opx = ps_pool.tile([P, 512], f32, tag="ps",
                                       name=f"op{b}{tb}{half}")
                    nc.tensor.matmul(opx,
                                     S["attnT"][:, tb * 128:(tb + 1) * 128],
                                     woutT[:, csl], start=True, stop=True)
                    nc.vector.tensor_copy(osb[:, csl], opx)
                nc.sync.dma_start(
                    out_d[b * T + tb * 128:b * T + (tb + 1) * 128, :], osb)

            def emit_norm(b, qc, S):
                # normalize attnT[:, qc] by 1/l via PE broadcast + DVE mul
                qsl = slice(qc * CW, (qc + 1) * CW)
                rb = ps_pool.tile([P, CW], f32, tag="ps",
                                  name=f"rb{b}{qc}")
                nc.tensor.matmul(rb, sel2[:, :], S["rl2"][:, qsl],
                                 start=True, stop=True)
                nc.vector.tensor_mul(S["attnT"][:, qsl], S["attnT"][:, qsl],
                                     rb)

            def emit_qc(b, qc, S):
                qsl = slice(qc * CW, (qc + 1) * CW)
                nk = 4 * qc + 4
                qT, kT = S["qT"], S["kT"]
                pv = [pv_pool.tile([P, CW], f32, tag="pv",
                                   name=f"pv{b}{qc}{h}") for h in range(2)]
                opq = deque(range(4 * (qc - 1), 4 * qc)) if qc > 0 else \
                    deque()
                for kb in range(nk):
                    off = max(0, (kb - 4 * qc) * 128)
                    st = st_pool.tile([P, 2, CW], f32, tag="st",
                                      name=f"st{b}{qc}{kb}")
                    pt = pt_pool.tile([P, 2, CW], bf16, tag="pt",
                                      name=f"pt{b}{qc}{kb}")
                    for h in range(2):
                        hs = h * 64
                        nc.tensor.matmul(
                            st[:, h, off:CW],
                            kT[hs:hs + 64, kb * 128:(kb + 1) * 128],
                            qT[hs:hs + 64, qc * CW + off:(qc + 1) * CW],
                            start=True, stop=True)
                    nc.scalar.activation(pt[:, :, off:CW], st[:, :, off:CW],
                                         Exp, scale=SCALE)
                    if kb == 0 and qc > 0:
                        emit_norm(b, qc - 1, S)
                    if kb >= 4 * qc:
                        for h in range(2):
                            nc.gpsimd.tensor_mul(
                                pt[:, h, off:off + 128],
                                pt[:, h, off:off + 128], maskT)
                    drain(1)
                    for h in range(2):
                        nc.tensor.matmul(
                            pv[h][:65, off:CW], S["vb"][:, kb, h, :],
                            pt[:, h, off:CW],
                            start=(kb == 0), stop=(kb == nk - 1),
                            skip_group_check=True)
                    if opq and kb % 2 == 1:
                        emit_oproj(b, opq.popleft(), S)
                # drain denominators + unnormalized attnT
                for h in range(2):
                    hs = h * 64
                    nc.vector.tensor_copy(S["l2"][hs:hs + 1, qsl],
                                          pv[h][64:65, :])
                    nc.vector.tensor_copy(S["attnT"][hs:hs + 64, qsl],
                                          pv[h][0:64, :])
                nc.vector.reciprocal_approx_fast(S["rl2"][:, qsl],
                                                 S["l2"][:, qsl])
                while opq:
                    emit_oproj(b, opq.popleft(), S)
                flush()

            states = [make_state(b) for b in range(B)]

            for f in make_chunk_quanta(0, 0, states[0]):
                f()
            fillers.extend(make_chunk_quanta(0, 1, states[0]))

            # (batch, chunk) to queue at the start of each qc step
            nxt = deque([(0, 2), (0, 3), (1, 0), (1, 1),
                         (1, 2), (1, 3), None, None])
            for b in range(B):
                for qc in range(4):
                    nx = nxt.popleft()
                    if nx is not None:
                        fillers.extend(
                            make_chunk_quanta(nx[0], nx[1], states[nx[0]]))
                    emit_qc(b, qc, states[b])
                # batch tail: normalize qc=3 and emit its out-projection
                emit_norm(b, 3, states[b])
                for tb in range(12, 16):
                    emit_oproj(b, tb, states[b])

    nc.compile()
    return nc


def _get_compiled():
    global _COMPILED
    if _COMPILED is None:
        _COMPILED = _build()
    return _COMPILED


def make_core_inputs(x, w_qkv, w_out):
    """Host-side shard prep: returns list of per-core input dicts."""
    xf = np.asarray(x, dtype=np.float32).reshape(BT, C)
    xT = np.ascontiguousarray(
        xf.T.reshape(8, P, BT).transpose(1, 0, 2)).astype(_bf16)

    maskT = np.zeros((P, P), dtype=_bf16)
    kk, qq = np.meshgrid(np.arange(P), np.arange(P), indexing="ij")
    maskT[kk <= qq] = 1.0

    sel2 = np.zeros((65, P), dtype=np.float32)
    sel2[0, 0:64] = 1.0
    sel2[64, 64:128] = 1.0

    ident = np.eye(P, dtype=_bf16)

    w_qkv = np.asarray(w_qkv, dtype=np.float32)
    w_out = np.asarray(w_out, dtype=np.float32)

    ins = []
    for core in range(NCORES):
        r0 = 2 * core * D
        wsel = np.concatenate([
            w_qkv[r0:r0 + 128],
            w_qkv[C + r0:C + r0 + 128],
            w_qkv[2 * C + r0:2 * C + r0 + 128],
        ], axis=0)  # [384, 1024]
        wqkvT = np.ascontiguousarray(
            wsel.T.reshape(8, P, 384).transpose(1, 0, 2)).astype(_bf16)
        woutT = np.ascontiguousarray(
            w_out[:, core * P:(core + 1) * P].T).astype(_bf16)
        ins.append({
            "xT": xT,
            "wqkvT": wqkvT,
            "woutT": woutT,
            "maskT": maskT,
            "sel2": sel2,
            "ident": ident,
        })
    return ins


def kernel(x, w_qkv, w_out):
    global LAST_RESULTS
    from concourse.bass_utils import run_bass_kernel_spmd

    nc = _get_compiled()
    ins = make_core_inputs(x, w_qkv, w_out)
    trace = bool(os.environ.get("KERNEL_TRACE"))
    res = run_bass_kernel_spmd(nc, ins, core_ids=list(range(NCORES)),
                               trace=trace)
    LAST_RESULTS = res
    out = np.zeros((BT, C), dtype=np.float32)
    for r in res.results:
        out += np.asarray(r["out"], dtype=np.float32)
    return out.reshape(B, T, C)


# revision 27
# speedup vs baseline: 1.2704x; 1.0242x over previous
"""Causal self-attention (B=2, T=2048, C=1024, 16 heads x 64) on 8 TRN2 cores.

Sharding: tensor-parallel over heads (2 heads/core). Each core computes its
heads' QKV projection, causal attention, and a partial output projection
(contraction over its 128 attn columns); the host sums the 8 partials.

v3 (chunk-pipelined, engine-balanced):
  - x DMA'd per 512-token chunk; chunk n+1's Q/K/V projection matmuls are
    emitted as PE fillers inside attention qc=n, so the exp-gated gaps get
    PE work, the input DMA overlaps compute (no 27us head), and the PE
    never idles long enough for the HAM clock-throttle to re-arm.
  - Scores PSUM tiles are bf16 (1 bank), both heads in one tile per
    k-block; exp is trimmed to the live causal range on diagonal blocks.
  - Causal mask multiplies run on the (otherwise idle) GpSimd engine;
    all PSUM->SBUF evacuation copies run on Vector; Scalar does exp only.
  - Output projection is one N=1024 bf16 matmul per token block into a
    single PSUM bank + one Vector copy + bf16 DMA out (host sums in f32).
  - V transposed per chunk (PE transpose); both heads' slices land in one
    strided Vector copy per token block.
"""

import os
from collections import deque

import numpy as np
import ml_dtypes

B = 2
T = 2048
C = 1024
N_HEADS = 16
D = 64
NCORES = 8
P = 128
BT = B * T
SCALE = D ** -0.5
NCH = 4          # 512-token chunks per batch
CW = T // NCH    # chunk width (= qc width)

_bf16 = ml_dtypes.bfloat16

_COMPILED = None
LAST_RESULTS = None  # stashed BassKernelResults for test harness introspection


def _build():
    import concourse.bass as bass
    import concourse.mybir as mybir
    import concourse.tile as tile
    from concourse import bacc

    f32 = mybir.dt.float32
    bf16 = mybir.dt.bfloat16

    nc = bacc.Bacc("TRN2", target_bir_lowering=False, debug=False,
                   num_devices=NCORES)

    xT_d = nc.dram_tensor("xT", [P, 8, BT], bf16, kind="ExternalInput")
    wqkvT_d = nc.dram_tensor("wqkvT", [P, 8, 384], bf16, kind="ExternalInput")
    woutT_d = nc.dram_tensor("woutT", [P, C], bf16, kind="ExternalInput")
    maskT_d = nc.dram_tensor("maskT", [P, P], bf16, kind="ExternalInput")
    sel2_d = nc.dram_tensor("sel2", [65, P], bf16, kind="ExternalInput")
    ident_d = nc.dram_tensor("ident", [P, P], bf16, kind="ExternalInput")
    out_d = nc.dram_tensor("out", [BT, C], bf16, kind="ExternalOutput")

    Exp = mybir.ActivationFunctionType.Exp

    with tile.TileContext(nc) as tc:
        with (
            tc.tile_pool(name="const", bufs=1) as const_pool,
            tc.tile_pool(name="xn", bufs=3) as xn_pool,
            tc.tile_pool(name="seq", bufs=2) as seq_pool,
            tc.tile_pool(name="vtn", bufs=2) as vtn_pool,
            tc.tile_pool(name="pt", bufs=4) as pt_pool,
            tc.tile_pool(name="rlb", bufs=2) as rlb_pool,
            tc.tile_pool(name="osb", bufs=3) as osb_pool,
            tc.tile_pool(name="st", bufs=2, space="PSUM") as st_pool,
            tc.tile_pool(name="pv", bufs=2, space="PSUM") as pv_pool,
            tc.tile_pool(name="ps", bufs=2, space="PSUM") as ps_pool,
        ):
            wqkvT = const_pool.tile([P, 8, 384], bf16, tag="wqkvT")
            ident = const_pool.tile([P, P], bf16, tag="ident")
            maskT = const_pool.tile([P, P], bf16, tag="maskT")
            sel2 = const_pool.tile([65, P], bf16, tag="sel2")
            woutT = const_pool.tile([P, C], bf16, tag="woutT")
            # v-slice of the weights + ident first so chunk 0's V projection
            # can start as early as possible; the rest follows behind.
            nc.sync.dma_start(wqkvT[:, :, 256:384], wqkvT_d[:, :, 256:384])
            nc.sync.dma_start(ident, ident_d[:])

            fillers = deque()

            def drain(k):
                n = 0
                while fillers and n < k:
                    fillers.popleft()()
                    n += 1

            def flush():
                drain(len(fillers))

            def make_state(b):
                S = {}
                S["qT"] = seq_pool.tile([P, T], bf16, tag="qT",
                                        name=f"qT{b}")
                S["kT"] = seq_pool.tile([P, T], bf16, tag="kT",
                                        name=f"kT{b}")
                S["attnT"] = seq_pool.tile([P, T], bf16, tag="attnT",
                                           name=f"attnT{b}")
                S["vb"] = seq_pool.tile([P, 16, 2, 65], bf16, tag="vb",
                                        name=f"vb{b}")
                nc.vector.memset(S["vb"][:, :, :, 64], 1.0)
                S["l2"] = seq_pool.tile([65, T], f32, tag="l2",
                                        name=f"l2{b}")
                S["rl2"] = seq_pool.tile([65, T], f32, tag="rl2",
                                         name=f"rl2{b}")
                # rows 1-63 feed zero sel2 rows; 1.0 keeps 1/x finite there
                nc.vector.memset(S["l2"], 1.0)
                return S

            def make_chunk_quanta(b, n, S):
                """Queue chunk n's DMA now; return PE quanta closures."""
                xn = xn_pool.tile([P, 8, CW], bf16, tag="xn",
                                  name=f"x{b}_{n}")
                nc.sync.dma_start(
                    xn, xT_d[:, :, b * T + n * CW:b * T + (n + 1) * CW])
                nsl = slice(n * CW, (n + 1) * CW)
                ps_tiles = {}

                def proj_half(key, fsl, c0):
                    def f():
                        if c0 == 0:
                            ps_tiles[key] = ps_pool.tile(
                                [P, CW], f32, tag="ps",
                                name=f"ps{key}{b}{n}")
                        ps = ps_tiles[key]
                        for c in range(c0, c0 + 4):
                            nc.tensor.matmul(ps, wqkvT[:, c, fsl],
                                             xn[:, c, :],
                                             start=(c == 0), stop=(c == 7))
                    return f

                def v_fin():
                    vtn = vtn_pool.tile([P, CW], bf16, tag="vtn",
                                        name=f"vtn{b}{n}")
                    S["vtn"] = vtn
                    nc.vector.tensor_copy(vtn, ps_tiles["v"])

                def t_pair(j0):
                    def f():
                        for j in (j0, j0 + 1):
                            tp = ps_pool.tile([P, 2, 64], bf16, tag="ps",
                                              name=f"tp{b}{n}{j}")
                            nc.tensor.transpose(
                                tp[:, :, :],
                                S["vtn"][:, j * 128:(j + 1) * 128], ident)
                            nc.vector.tensor_copy(
                                S["vb"][:, 4 * n + j, :, 0:64], tp[:, :, :])
                    return f

                def q_fin():
                    nc.scalar.copy(S["qT"][:, nsl], ps_tiles["q"])

                def k_fin():
                    nc.scalar.copy(S["kT"][:, nsl], ps_tiles["k"])

                qv1 = proj_half("v", slice(256, 384), 0)
                qv2 = proj_half("v", slice(256, 384), 4)
                qq1 = proj_half("q", slice(0, 128), 0)
                qq2 = proj_half("q", slice(0, 128), 4)
                qk1 = proj_half("k", slice(128, 256), 0)
                qk2 = proj_half("k", slice(128, 256), 4)
                return [qv1, lambda: (qv2(), v_fin()), t_pair(0), t_pair(2),
                        qq1, lambda: (qq2(), q_fin()),
                        qk1, lambda: (qk2(), k_fin())]

            def emit_oproj(b, tb, S, tail=False):
                osb = osb_pool.tile([P, C], bf16, tag="osb",
                                    name=f"osb{b}{tb}")
                rows = slice(b * T + tb * 128, b * T + (tb + 1) * 128)
                for half in range(2):
                    csl = slice(half * 512, (half + 1) * 512)
                    opx = ps_pool.tile([P, 512], f32, tag="ps",
                                       name=f"op{b}{tb}{half}")
                    nc.tensor.matmul(opx,
                                     S["attnT"][:, tb * 128:(tb + 1) * 128],
                                     woutT[:, csl], start=True, stop=True)
                    if tail and half == 0:
                        # spread tail evacuation over Scalar+Vector and two
                        # DMA queues to shorten the serial endgame
                        nc.scalar.copy(osb[:, csl], opx)
                        nc.sync.dma_start(out_d[rows, csl], osb[:, csl])
                    else:
                        nc.vector.tensor_copy(osb[:, csl], opx)
                        if tail:
                            nc.sync.dma_start(out_d[rows, csl], osb[:, csl])
                if not tail:
                    nc.sync.dma_start(out_d[rows, :], osb)

            def emit_norm(b, qc, S):
                # normalize attnT[:, qc] by 1/l via PE broadcast + DVE mul
                qsl = slice(qc * CW, (qc + 1) * CW)
                rb = ps_pool.tile([P, CW], f32, tag="ps",
                                  name=f"rb{b}{qc}")
                nc.tensor.matmul(rb, sel2[:, :], S["rlb"],
                                 start=True, stop=True)
                nc.vector.tensor_mul(S["attnT"][:, qsl], S["attnT"][:, qsl],
                                     rb)

            def emit_qc(b, qc, S):
                qsl = slice(qc * CW, (qc + 1) * CW)
                nk = 4 * qc + 4
                qT, kT = S["qT"], S["kT"]
                pv = [pv_pool.tile([P, CW], f32, tag="pv",
                                   name=f"pv{b}{qc}{h}") for h in range(2)]
                opq = deque(range(4 * (qc - 1), 4 * qc)) if qc > 0 else \
                    deque()
                for kb in range(nk):
                    off = max(0, (kb - 4 * qc) * 128)
                    st = st_pool.tile([P, 2, CW], f32, tag="st",
                                      name=f"st{b}{qc}{kb}")
                    pt = pt_pool.tile([P, 2, CW], bf16, tag="pt",
                                      name=f"pt{b}{qc}{kb}")
                    for h in range(2):
                        hs = h * 64
                        nc.tensor.matmul(
                            st[:, h, off:CW],
                            kT[hs:hs + 64, kb * 128:(kb + 1) * 128],
                            qT[hs:hs + 64, qc * CW + off:(qc + 1) * CW],
                            start=True, stop=True)
                    nc.scalar.activation(pt[:, :, off:CW], st[:, :, off:CW],
                                         Exp, scale=SCALE)
                    if kb == 0 and qc > 0:
                        emit_norm(b, qc - 1, S)
                    if kb >= 4 * qc:
                        for h in range(2):
                            nc.gpsimd.tensor_mul(
                                pt[:, h, off:off + 128],
                                pt[:, h, off:off + 128], maskT)
                    drain(1)
                    for h in range(2):
                        nc.tensor.matmul(
                            pv[h][:65, off:CW], S["vb"][:, kb, h, :],
                            pt[:, h, off:CW],
                            start=(kb == 0), stop=(kb == nk - 1),
                            skip_group_check=True)
                    if opq and kb % 2 == 1:
                        emit_oproj(b, opq.popleft(), S)
                # drain denominators + unnormalized attnT
                for h in range(2):
                    hs = h * 64
                    nc.vector.tensor_copy(S["l2"][hs:hs + 1, qsl],
                                          pv[h][64:65, :])
                    nc.vector.tensor_copy(S["attnT"][hs:hs + 64, qsl],
                                          pv[h][0:64, :])
                nc.vector.reciprocal_approx_fast(S["rl2"][:, qsl],
                                                 S["l2"][:, qsl])
                # bf16 copy of 1/l so the broadcast matmul avoids fp32 mode
                rlb = rlb_pool.tile([65, CW], bf16, tag="rlb",
                                    name=f"rlb{b}{qc}")
                nc.vector.tensor_copy(rlb, S["rl2"][:, qsl])
                S["rlb"] = rlb
                while opq:
                    emit_oproj(b, opq.popleft(), S)
                flush()

            states = [make_state(b) for b in range(B)]

            chunk0 = make_chunk_quanta(0, 0, states[0])
            nc.sync.dma_start(wqkvT[:, :, 0:256], wqkvT_d[:, :, 0:256])
            nc.sync.dma_start(maskT, maskT_d[:])
            nc.sync.dma_start(sel2, sel2_d[:])
            nc.sync.dma_start(woutT, woutT_d[:])
            for f in chunk0:
                f()
            fillers.extend(make_chunk_quanta(0, 1, states[0]))

            # (batch, chunk) to queue at the start of each qc step
            nxt = deque([(0, 2), (0, 3), (1, 0), (1, 1),
                         (1, 2), (1, 3), None, None])
            for b in range(B):
                for qc in range(4):
                    nx = nxt.popleft()
                    if nx is not None:
                        fillers.extend(
                            make_chunk_quanta(nx[0], nx[1], states[nx[0]]))
                    emit_qc(b, qc, states[b])
                # batch tail: normalize qc=3 and emit its out-projection
                emit_norm(b, 3, states[b])
                for tb in range(12, 16):
                    emit_oproj(b, tb, states[b], tail=(b == B - 1))

    nc.compile()
    return nc


def _get_compiled():
    global _COMPILED
    if _COMPILED is None:
        _COMPILED = _build()
    return _COMPILED


def make_core_inputs(x, w_qkv, w_out):
    """Host-side shard prep: returns list of per-core input dicts."""
    xf = np.asarray(x, dtype=np.float32).reshape(BT, C)
    xT = np.ascontiguousarray(
        xf.T.reshape(8, P, BT).transpose(1, 0, 2)).astype(_bf16)

    maskT = np.zeros((P, P), dtype=_bf16)
    kk, qq = np.meshgrid(np.arange(P), np.arange(P), indexing="ij")
    maskT[kk <= qq] = 1.0

    sel2 = np.zeros((65, P), dtype=_bf16)
    sel2[0, 0:64] = 1.0
    sel2[64, 64:128] = 1.0

    ident = np.eye(P, dtype=_bf16)

    w_qkv = np.asarray(w_qkv, dtype=np.float32)
    w_out = np.asarray(w_out, dtype=np.float32)

    ins = []
    for core in range(NCORES):
        r0 = 2 * core * D
        wsel = np.concatenate([
            w_qkv[r0:r0 + 128],
            w_qkv[C + r0:C + r0 + 128],
            w_qkv[2 * C + r0:2 * C + r0 + 128],
        ], axis=0)  # [384, 1024]
        wqkvT = np.ascontiguousarray(
            wsel.T.reshape(8, P, 384).transpose(1, 0, 2)).astype(_bf16)
        woutT = np.ascontiguousarray(
            w_out[:, core * P:(core + 1) * P].T).astype(_bf16)
        ins.append({
            "xT": xT,
            "wqkvT": wqkvT,
            "woutT": woutT,
            "maskT": maskT,
            "sel2": sel2,
            "ident": ident,
        })
    return ins


def kernel(x, w_qkv, w_out):
    global LAST_RESULTS
    from concourse.bass_utils import run_bass_kernel_spmd

    nc = _get_compiled()
    ins = make_core_inputs(x, w_qkv, w_out)
    trace = bool(os.environ.get("KERNEL_TRACE"))
    res = run_bass_kernel_spmd(nc, ins, core_ids=list(range(NCORES)),
                               trace=trace)
    LAST_RESULTS = res
    out = np.zeros((BT, C), dtype=np.float32)
    for r in res.results:
        out += np.asarray(r["out"], dtype=np.float32)
    return out.reshape(B, T, C)


# revision 32
# speedup vs baseline: 1.3487x; 1.0616x over previous
"""Causal self-attention (B=2, T=2048, C=1024, 16 heads x 64) on 8 TRN2 cores.

Sharding: tensor-parallel over heads (2 heads/core). Each core computes its
heads' QKV projection, causal attention, and a partial output projection
(contraction over its 128 attn columns); the host sums the 8 partials.

v3 (chunk-pipelined, engine-balanced):
  - x DMA'd per 512-token chunk; chunk n+1's Q/K/V projection matmuls are
    emitted as PE fillers inside attention qc=n, so the exp-gated gaps get
    PE work, the input DMA overlaps compute (no 27us head), and the PE
    never idles long enough for the HAM clock-throttle to re-arm.
  - Scores PSUM tiles are bf16 (1 bank), both heads in one tile per
    k-block; exp is trimmed to the live causal range on diagonal blocks.
  - Causal mask multiplies run on the (otherwise idle) GpSimd engine;
    all PSUM->SBUF evacuation copies run on Vector; Scalar does exp only.
  - Output projection is one N=1024 bf16 matmul per token block into a
    single PSUM bank + one Vector copy + bf16 DMA out (host sums in f32).
  - V transposed per chunk (PE transpose); both heads' slices land in one
    strided Vector copy per token block.
"""

import os
from collections import deque

import numpy as np
import ml_dtypes

B = 2
T = 2048
C = 1024
N_HEADS = 16
D = 64
NCORES = 8
P = 128
BT = B * T
SCALE = D ** -0.5
NCH = 4          # 512-token chunks per batch
CW = T // NCH    # chunk width (= qc width)

_bf16 = ml_dtypes.bfloat16

_COMPILED = None
LAST_RESULTS = None  # stashed BassKernelResults for test harness introspection


def _build():
    import concourse.bass as bass
    import concourse.mybir as mybir
    import concourse.tile as tile
    from concourse import bacc

    f32 = mybir.dt.float32
    bf16 = mybir.dt.bfloat16

    nc = bacc.Bacc("TRN2", target_bir_lowering=False, debug=False,
                   num_devices=NCORES)

    xT_d = nc.dram_tensor("xT", [P, 8, BT], bf16, kind="ExternalInput")
    wqkvT_d = nc.dram_tensor("wqkvT", [P, 8, 384], bf16, kind="ExternalInput")
    woutT_d = nc.dram_tensor("woutT", [P, C], bf16, kind="ExternalInput")
    maskT_d = nc.dram_tensor("maskT", [P, P], bf16, kind="ExternalInput")
    sel2_d = nc.dram_tensor("sel2", [65, P], bf16, kind="ExternalInput")
    ident_d = nc.dram_tensor("ident", [P, P], bf16, kind="ExternalInput")
    out_d = nc.dram_tensor("out", [BT, C], bf16, kind="ExternalOutput")

    Exp = mybir.ActivationFunctionType.Exp

    with tile.TileContext(nc) as tc:
        with (
            tc.tile_pool(name="const", bufs=1) as const_pool,
            tc.tile_pool(name="xn", bufs=3) as xn_pool,
            tc.tile_pool(name="seq", bufs=2) as seq_pool,
            tc.tile_pool(name="vtn", bufs=2) as vtn_pool,
            tc.tile_pool(name="pt", bufs=4) as pt_pool,
            tc.tile_pool(name="rlb", bufs=2) as rlb_pool,
            tc.tile_pool(name="osb", bufs=3) as osb_pool,
            tc.tile_pool(name="st", bufs=2, space="PSUM") as st_pool,
            tc.tile_pool(name="pv", bufs=2, space="PSUM") as pv_pool,
            tc.tile_pool(name="ps", bufs=2, space="PSUM") as ps_pool,
        ):
            wqkvT = const_pool.tile([P, 8, 384], bf16, tag="wqkvT")
            ident = const_pool.tile([P, P], bf16, tag="ident")
            maskT = const_pool.tile([P, P], bf16, tag="maskT")
            sel2 = const_pool.tile([65, P], bf16, tag="sel2")
            woutT = const_pool.tile([P, C], bf16, tag="woutT")
            # v-slice of the weights + ident first so chunk 0's V projection
            # can start as early as possible; the rest follows behind.
            nc.sync.dma_start(wqkvT[:, :, 256:384], wqkvT_d[:, :, 256:384])
            nc.sync.dma_start(ident, ident_d[:])

            fillers = deque()

            def drain(k):
                n = 0
                while fillers and n < k:
                    fillers.popleft()()
                    n += 1

            def flush():
                drain(len(fillers))

            def make_state(b):
                S = {}
                S["qT"] = seq_pool.tile([P, T], bf16, tag="qT",
                                        name=f"qT{b}")
                S["kT"] = seq_pool.tile([P, T], bf16, tag="kT",
                                        name=f"kT{b}")
                S["attnT"] = seq_pool.tile([P, T], bf16, tag="attnT",
                                           name=f"attnT{b}")
                S["vb"] = seq_pool.tile([P, 16, 2, 65], bf16, tag="vb",
                                        name=f"vb{b}")
                nc.vector.memset(S["vb"][:, :, :, 64], 1.0)
                S["l2"] = seq_pool.tile([65, T], f32, tag="l2",
                                        name=f"l2{b}")
                S["rl2"] = seq_pool.tile([65, T], f32, tag="rl2",
                                         name=f"rl2{b}")
                # rows 1-63 feed zero sel2 rows; 1.0 keeps 1/x finite there
                nc.vector.memset(S["l2"], 1.0)
                return S

            def make_chunk_quanta(b, n, S):
                """Queue chunk n's DMA now; return PE quanta closures."""
                xn = xn_pool.tile([P, 8, CW], bf16, tag="xn",
                                  name=f"x{b}_{n}")
                tsl = slice(b * T + n * CW, b * T + (n + 1) * CW)
                nc.sync.dma_start(xn[:, 0:4, :], xT_d[:, 0:4, tsl])
                nc.sync.dma_start(xn[:, 4:8, :], xT_d[:, 4:8, tsl])
                nsl = slice(n * CW, (n + 1) * CW)
                ps_tiles = {}

                def proj_pair(key, fsl, c0):
                    def f():
                        if c0 == 0:
                            ps_tiles[key] = ps_pool.tile(
                                [P, CW], f32, tag="ps",
                                name=f"ps{key}{b}{n}")
                        ps = ps_tiles[key]
                        for c in range(c0, c0 + 2):
                            nc.tensor.matmul(ps, wqkvT[:, c, fsl],
                                             xn[:, c, :],
                                             start=(c == 0), stop=(c == 7))
                    return f

                def v_fin():
                    vtn = vtn_pool.tile([P, CW], bf16, tag="vtn",
                                        name=f"vtn{b}{n}")
                    S["vtn"] = vtn
                    nc.vector.tensor_copy(vtn, ps_tiles["v"])

                def t_one(j):
                    def f():
                        tp = ps_pool.tile([P, 2, 64], bf16, tag="ps",
                                          name=f"tp{b}{n}{j}")
                        nc.tensor.transpose(
                            tp[:, :, :],
                            S["vtn"][:, j * 128:(j + 1) * 128], ident)
                        nc.vector.tensor_copy(
                            S["vb"][:, 4 * n + j, :, 0:64], tp[:, :, :])
                    return f

                def q_fin():
                    nc.scalar.copy(S["qT"][:, nsl], ps_tiles["q"])

                def k_fin():
                    nc.scalar.copy(S["kT"][:, nsl], ps_tiles["k"])

                def chain(f, g):
                    return lambda: (f(), g())

                vsl, qsl_, ksl = (slice(256, 384), slice(0, 128),
                                  slice(128, 256))
                qs = [proj_pair("v", vsl, c) for c in (0, 2, 4)]
                qs.append(chain(proj_pair("v", vsl, 6), v_fin))
                qs.extend(t_one(j) for j in range(4))
                qs.extend(proj_pair("q", qsl_, c) for c in (0, 2, 4))
                qs.append(chain(proj_pair("q", qsl_, 6), q_fin))
                qs.extend(proj_pair("k", ksl, c) for c in (0, 2, 4))
                qs.append(chain(proj_pair("k", ksl, 6), k_fin))
                return qs

            def emit_oproj(b, tb, S, tail=False):
                osb = osb_pool.tile([P, C], bf16, tag="osb",
                                    name=f"osb{b}{tb}")
                rows = slice(b * T + tb * 128, b * T + (tb + 1) * 128)
                for half in range(2):
                    csl = slice(half * 512, (half + 1) * 512)
                    opx = ps_pool.tile([P, 512], f32, tag="ps",
                                       name=f"op{b}{tb}{half}")
                    nc.tensor.matmul(opx,
                                     S["attnT"][:, tb * 128:(tb + 1) * 128],
                                     woutT[:, csl], start=True, stop=True)
                    if tail and half == 0:
                        # spread tail evacuation over Scalar+Vector and two
                        # DMA queues to shorten the serial endgame
                        nc.scalar.copy(osb[:, csl], opx)
                        nc.sync.dma_start(out_d[rows, csl], osb[:, csl])
                    else:
                        nc.vector.tensor_copy(osb[:, csl], opx)
                        if tail:
                            nc.sync.dma_start(out_d[rows, csl], osb[:, csl])
                if not tail:
                    nc.sync.dma_start(out_d[rows, :], osb)

            def emit_norm(b, qc, S):
                # normalize attnT[:, qc] by 1/l via PE broadcast + DVE mul
                qsl = slice(qc * CW, (qc + 1) * CW)
                rb = ps_pool.tile([P, CW], f32, tag="ps",
                                  name=f"rb{b}{qc}")
                nc.tensor.matmul(rb, sel2[:, :], S["rlb"],
                                 start=True, stop=True)
                nc.vector.tensor_mul(S["attnT"][:, qsl], S["attnT"][:, qsl],
                                     rb)

            def emit_qc(b, qc, S):
                qsl = slice(qc * CW, (qc + 1) * CW)
                nk = 4 * qc + 4
                qT, kT = S["qT"], S["kT"]
                pv = [pv_pool.tile([P, CW], f32, tag="pv",
                                   name=f"pv{b}{qc}{h}") for h in range(2)]
                opq = deque(range(4 * (qc - 1), 4 * qc)) if qc > 0 else \
                    deque()
                for kb in range(nk):
                    off = max(0, (kb - 4 * qc) * 128)
                    st = st_pool.tile([P, 2, CW], f32, tag="st",
                                      name=f"st{b}{qc}{kb}")
                    pt = pt_pool.tile([P, 2, CW], bf16, tag="pt",
                                      name=f"pt{b}{qc}{kb}")
                    for h in range(2):
                        hs = h * 64
                        nc.tensor.matmul(
                            st[:, h, off:CW],
                            kT[hs:hs + 64, kb * 128:(kb + 1) * 128],
                            qT[hs:hs + 64, qc * CW + off:(qc + 1) * CW],
                            start=True, stop=True)
                    if kb >= 4 * qc:
                        # per-head exp so head 0's mask+PV overlap head 1's
                        for h in range(2):
                            nc.scalar.activation(pt[:, h, off:CW],
                                                 st[:, h, off:CW],
                                                 Exp, scale=SCALE)
                            nc.gpsimd.tensor_mul(
                                pt[:, h, off:off + 128],
                                pt[:, h, off:off + 128], maskT)
                    else:
                        nc.scalar.activation(pt[:, :, off:CW],
                                             st[:, :, off:CW],
                                             Exp, scale=SCALE)
                    if kb == 0 and qc > 0:
                        emit_norm(b, qc - 1, S)
                    drain(1)
                    for h in range(2):
                        nc.tensor.matmul(
                            pv[h][:65, off:CW], S["vb"][:, kb, h, :],
                            pt[:, h, off:CW],
                            start=(kb == 0), stop=(kb == nk - 1),
                            skip_group_check=True)
                    if opq and kb % 2 == 1:
                        emit_oproj(b, opq.popleft(), S)
                # drain denominators (Vector) + attnT (Scalar) in parallel
                for h in range(2):
                    hs = h * 64
                    nc.vector.tensor_copy(S["l2"][hs:hs + 1, qsl],
                                          pv[h][64:65, :])
                    nc.scalar.copy(S["attnT"][hs:hs + 64, qsl],
                                   pv[h][0:64, :])
                nc.vector.reciprocal_approx_fast(S["rl2"][:, qsl],
                                                 S["l2"][:, qsl])
                # bf16 copy of 1/l so the broadcast matmul avoids fp32 mode
                rlb = rlb_pool.tile([65, CW], bf16, tag="rlb",
                                    name=f"rlb{b}{qc}")
                nc.vector.tensor_copy(rlb, S["rl2"][:, qsl])
                S["rlb"] = rlb
                while opq:
                    emit_oproj(b, opq.popleft(), S)
                flush()

            states = [make_state(b) for b in range(B)]

            chunk0 = make_chunk_quanta(0, 0, states[0])
            nc.sync.dma_start(wqkvT[:, :, 0:256], wqkvT_d[:, :, 0:256])
            nc.sync.dma_start(maskT, maskT_d[:])
            nc.sync.dma_start(sel2, sel2_d[:])
            nc.sync.dma_start(woutT, woutT_d[:])
            for f in chunk0:
                f()
            fillers.extend(make_chunk_quanta(0, 1, states[0]))

            # (batch, chunk) to queue at the start of each qc step
            nxt = deque([(0, 2), (0, 3), (1, 0), (1, 1),
                         (1, 2), (1, 3), None, None])
            for b in range(B):
                for qc in range(4):
                    nx = nxt.popleft()
                    if nx is not None:
                        fillers.extend(
                            make_chunk_quanta(nx[0], nx[1], states[nx[0]]))
                    emit_qc(b, qc, states[b])
                # batch tail: normalize qc=3 and emit its out-projection
                emit_norm(b, 3, states[b])
                for tb in range(12, 16):
                    emit_oproj(b, tb, states[b], tail=(b == B - 1))

    nc.compile()
    return nc


def _get_compiled():
    global _COMPILED
    if _COMPILED is None:
        _COMPILED = _build()
    return _COMPILED


def make_core_inputs(x, w_qkv, w_out):
    """Host-side shard prep: returns list of per-core input dicts."""
    xf = np.asarray(x, dtype=np.float32).reshape(BT, C)
    xT = np.ascontiguousarray(
        xf.T.reshape(8, P, BT).transpose(1, 0, 2)).astype(_bf16)

    maskT = np.zeros((P, P), dtype=_bf16)
    kk, qq = np.meshgrid(np.arange(P), np.arange(P), indexing="ij")
    maskT[kk <= qq] = 1.0

    sel2 = np.zeros((65, P), dtype=_bf16)
    sel2[0, 0:64] = 1.0
    sel2[64, 64:128] = 1.0

    ident = np.eye(P, dtype=_bf16)

    w_qkv = np.asarray(w_qkv, dtype=np.float32)
    w_out = np.asarray(w_out, dtype=np.float32)

    ins = []
    for core in range(NCORES):
        r0 = 2 * core * D
        wsel = np.concatenate([
            w_qkv[r0:r0 + 128],
            w_qkv[C + r0:C + r0 + 128],
            w_qkv[2 * C + r0:2 * C + r0 + 128],
        ], axis=0)  # [384, 1024]
        wqkvT = np.ascontiguousarray(
            wsel.T.reshape(8, P, 384).transpose(1, 0, 2)).astype(_bf16)
        woutT = np.ascontiguousarray(
            w_out[:, core * P:(core + 1) * P].T).astype(_bf16)
        ins.append({
            "xT": xT,
            "wqkvT": wqkvT,
            "woutT": woutT,
            "maskT": maskT,
            "sel2": sel2,
            "ident": ident,
        })
    return ins


def kernel(x, w_qkv, w_out):
    global LAST_RESULTS
    from concourse.bass_utils import run_bass_kernel_spmd

    nc = _get_compiled()
    ins = make_core_inputs(x, w_qkv, w_out)
    trace = bool(os.environ.get("KERNEL_TRACE"))
    res = run_bass_kernel_spmd(nc, ins, core_ids=list(range(NCORES)),
                               trace=trace)
    LAST_RESULTS = res
    out = np.zeros((BT, C), dtype=np.float32)
    for r in res.results:
        out += np.asarray(r["out"], dtype=np.float32)
    return out.reshape(B, T, C)
